# revision 3
# baseline (speedup 1.0000x reference)
"""DepthwiseSeparableDCNv2 for Trainium2 — self-contained 8-core SPMD Bass kernel.

kernel(**inputs) takes the full unsharded inputs and returns the full
[4, 256, 128, 128] float32 output. Sharding: 4 batch samples x 2 H-halves.

v2 pipeline per core (vs the v1 baseline):
  - gathers read a single unscaled pixel-major slab (80 rows + halo) from
    DRAM; the per-tap depthwise weight wk[c,k] is applied on-device with a
    2x-rate tensor_tensor against a partition-replicated wk tile, instead
    of shipping a 9-tap pre-scaled 37.7 MB image from the host.
  - one dma_gather per 4-row group covers all 9 taps (9216 indices).
  - the 36-term bilinear MAC per row is split between the DVE and Pool
    engines into two accumulators, which the PE transpose sums for free
    via PSUM accumulation.
  - pointwise conv + bias run batched over 4 rows; output is fp16.
"""
import numpy as np
import ml_dtypes
from contextlib import ExitStack

import concourse.bass as bass
from concourse import bacc
import concourse.mybir as mybir
from concourse.tile import TileContext
from concourse._compat import with_exitstack
from concourse import library_config

DT = mybir.dt
Alu = mybir.AluOpType
AF = mybir.ActivationFunctionType

B, C, H, W, O = 4, 128, 128, 128, 256
K2 = 9
ROWS = 64          # output rows per core
RB = 32            # idx-math batch rows
GG = 4             # rows per gather group
NG = RB // GG      # gather groups per batch
NIDX = GG * 2 * K2 * 128   # indices per merged gather instruction (9216)
SLAB_ROWS = 80     # slab rows held per core (64 out rows + 16 halo)
SLAB_U = SLAB_ROWS * 128   # slab units
N_DVE = 10         # MAC terms per row on DVE; remaining 36-N_DVE on Pool

CONS_W = 9 + 9 + 64 + 64 + 64 + 2 + 27  # 239

# scratch slot ids in the consolidated [128, NS, RB, 9] f32 tile
(S_MSK, S_WY, S_Y0S, S_Y1S, S_V0, S_V1, S_Y0C, S_Y1C, S_WX, S_X0S, S_X1S,
 S_XB, S_XB1, S_AS0, S_AS1, S_T0, S_T1, S_AWX, S_AWY, S_WY0M, S_WY1M,
 S_TMP) = range(22)
NS = 22
S_TYS = S_TMP   # tys -> txs -> adr share one slot (sequential lifetimes)
S_TXS = S_TMP
S_ADR = S_TMP
S_I0F = S_V0    # v0/v1 dead once wy0m/wy1m built
S_I1F = S_V1


def build_nc():
    nc = bacc.Bacc("TRN2", target_bir_lowering=False, debug=False,
                   num_devices=8, num_swdge_queues=4)
    xc = nc.dram_tensor("xc", [128, 66 * 130], DT.float16, kind="ExternalInput")
    xs = nc.dram_tensor("xs", [SLAB_U + 1, 128], DT.float16, kind="ExternalInput")
    woff = nc.dram_tensor("woff", [128, K2 * 27], DT.float16, kind="ExternalInput")
    wpw = nc.dram_tensor("wpw", [128, 256], DT.float16, kind="ExternalInput")
    idn = nc.dram_tensor("idn", [128, 128], DT.float16, kind="ExternalInput")
    wkr = nc.dram_tensor("wkr", [128, K2 * 256], DT.float16, kind="ExternalInput")
    cons = nc.dram_tensor("cons", [128, CONS_W], DT.float32, kind="ExternalInput")
    out = nc.dram_tensor("out", [256, ROWS, 128], DT.float16, kind="ExternalOutput")

    with TileContext(nc) as tc:
        _kernel(tc, xc, xs, woff, wpw, idn, wkr, cons, out)

    nc.compile()
    legalize_single_wait(nc)
    bass.Bass.finalize(nc)
    return nc


@with_exitstack
def _kernel(ctx: ExitStack, tc: TileContext, xc, xs, woff, wpw, idn, wkr,
            cons, out):
    nc = tc.nc

    cpool = ctx.enter_context(tc.tile_pool(name="const", bufs=1))
    XC = cpool.tile([128, 66 * 130], DT.float16)
    nc.sync.dma_start(XC[:], xc.ap())
    WOF = cpool.tile([128, K2, 27], DT.float16)
    nc.sync.dma_start(WOF[:], woff.ap())
    WPW = cpool.tile([128, 256], DT.float16)
    nc.sync.dma_start(WPW[:], wpw.ap())
    IDN = cpool.tile([128, 128], DT.float16)
    nc.sync.dma_start(IDN[:], idn.ap())
    WKR = cpool.tile([128, K2, 256], DT.float16)
    nc.sync.dma_start(WKR[:], wkr.ap())
    CON = cpool.tile([128, CONS_W], DT.float32)
    nc.sync.dma_start(CON[:], cons.ap())

    KY = CON[:, 0:9]           # ky + 16                  [128, 9]
    KX = CON[:, 9:18]          # w + kx + 16              [128, 9]
    HL = CON[:, 18:82]         # slab lo clamp per row    [128, 64]
    HH = CON[:, 82:146]        # slab hi clamp per row    [128, 64]
    HOF = CON[:, 146:210]      # slab unit offset per row [128, 64]
    BPW = CON[:, 210:212]      # fused pointwise bias     [128, 2]

    om_ps = ctx.enter_context(tc.tile_pool(name="omp", bufs=2, space="PSUM"))
    tr_ps = ctx.enter_context(tc.tile_pool(name="trp", bufs=2, space="PSUM"))
    pw_ps = ctx.enter_context(tc.tile_pool(name="pwp", bufs=2, space="PSUM"))
    oms_pool = ctx.enter_context(tc.tile_pool(name="oms", bufs=2))
    mpool = ctx.enter_context(tc.tile_pool(name="m", bufs=1))
    wpool = ctx.enter_context(tc.tile_pool(name="wp", bufs=2))
    ipool = ctx.enter_context(tc.tile_pool(name="ip", bufs=2))
    wrpool = ctx.enter_context(tc.tile_pool(name="wr", bufs=2))
    gpool = ctx.enter_context(tc.tile_pool(name="g", bufs=2))
    apool = ctx.enter_context(tc.tile_pool(name="acc", bufs=2))
    rpool = ctx.enter_context(tc.tile_pool(name="rt", bufs=2))
    opool = ctx.enter_context(tc.tile_pool(name="o", bufs=2))

    out_v = out.ap().rearrange("(oh o) r w -> o oh r w", oh=2)
    gidx = [0]
    nidx_reg = ctx.enter_context(nc.gpsimd.register("nidx"))
    nc.gpsimd.reg_mov(nidx_reg, NIDX)

    # MAC term split: first N_DVE per row on DVE, rest on Pool
    terms = [(k, c2, s2) for k in range(K2) for c2 in range(2)
             for s2 in range(2)]
    dve_terms = terms[:N_DVE]
    pool_terms = terms[N_DVE:]

    for bt in range(2):
        # ---- offset conv: om.T [w, 27] per row ----
        OMS = oms_pool.tile([128, RB, 27], DT.float32, tag="oms")
        for r in range(RB):
            om = om_ps.tile([128, 27], DT.float32, tag="om", name="om")
            pos = (bt * RB + r + 1) * 130 + 1
            for t in range(K2):
                ty, tx = t // 3, t % 3
                sh = (ty - 1) * 130 + (tx - 1)
                nc.tensor.matmul(om[:], XC[:, pos + sh: pos + sh + 128],
                                 WOF[:, t, :], start=(t == 0), stop=(t == 8))
            nc.scalar.activation(OMS[:, r, :], om[:], AF.Copy)
        # b_off (broadcast over rows)
        _bof = CON[:, 212:239]
        bof_b = bass.AP(tensor=_bof.tensor, offset=_bof.offset,
                        ap=[list(_bof.ap[0]), [0, RB], [1, 27]])
        nc.vector.tensor_tensor(OMS[:], OMS[:], bof_b, op=Alu.add)

        # ---- index / weight math ----
        SCR = mpool.tile([128, NS, RB, K2], DT.float32, tag="scr", name="scr")

        def s(i):
            return SCR[:, i]

        nc.scalar.activation(s(S_MSK), OMS[:, :, 18:27], AF.Sigmoid)

        offs = OMS[:, :, 0:18].rearrange("p r (k two) -> p two r k", two=2)
        dy, dx = offs[:, 0], offs[:, 1]

        def bc9(ap128x9):   # [128, 9] -> [128, RB, 9] broadcast over rows
            return bass.AP(tensor=ap128x9.tensor, offset=ap128x9.offset,
                           ap=[list(ap128x9.ap[0]), [0, RB], [1, 9]])

        def bcrow(ap128x64):  # [128, 64] row-consts -> [128, RB, 9] for batch bt
            sl = ap128x64[:, bt * RB:(bt + 1) * RB]
            return bass.AP(tensor=sl.tensor, offset=sl.offset,
                           ap=[list(sl.ap[0]), [1, RB], [0, 9]])

        KYb, KXb = bc9(KY), bc9(KX)
        HLb, HHb, HOFb = bcrow(HL), bcrow(HH), bcrow(HOF)
        v = nc.vector

        W4 = wpool.tile([128, 4, RB, K2], DT.float32, tag="w4")
        IAL = ipool.tile([128, NG, K2, 2, GG], DT.int16, tag="ial")
        WR = wrpool.tile([128, NG, K2, 2, GG, 8], DT.int16, tag="wr")

        v.tensor_tensor(s(S_TYS), dy, KYb, op=Alu.add)
        v.tensor_scalar(s(S_TYS), s(S_TYS), 0.0, None, Alu.max)
        # floor via the 2^23 magic number: RNE(x - 0.5) == floor(x) up to
        # integer ties, which bilinear continuity makes harmless
        v.tensor_scalar(s(S_Y0S), s(S_TYS), 8388607.5, 8388608.0,
                        Alu.add, Alu.subtract)
        v.tensor_tensor(s(S_WY), s(S_TYS), s(S_Y0S), op=Alu.subtract)
        v.tensor_scalar(s(S_Y1S), s(S_Y0S), 1.0, None, Alu.add)
        v.tensor_tensor(s(S_T0), s(S_Y0S), HLb, op=Alu.is_ge)
        v.tensor_tensor(s(S_T1), s(S_Y0S), HHb, op=Alu.is_le)
        v.tensor_tensor(s(S_V0), s(S_T0), s(S_T1), op=Alu.mult)
        v.tensor_tensor(s(S_T0), s(S_Y1S), HLb, op=Alu.is_ge)
        v.tensor_tensor(s(S_T1), s(S_Y1S), HHb, op=Alu.is_le)
        v.tensor_tensor(s(S_V1), s(S_T0), s(S_T1), op=Alu.mult)
        v.tensor_tensor(s(S_Y0C), s(S_Y0S), HLb, op=Alu.max)
        v.tensor_tensor(s(S_Y0C), s(S_Y0C), HHb, op=Alu.min)
        v.tensor_tensor(s(S_Y1C), s(S_Y1S), HLb, op=Alu.max)
        v.tensor_tensor(s(S_Y1C), s(S_Y1C), HHb, op=Alu.min)

        v.tensor_tensor(s(S_TXS), dx, KXb, op=Alu.add)
        v.tensor_scalar(s(S_TXS), s(S_TXS), 0.0, None, Alu.max)
        v.tensor_scalar(s(S_X0S), s(S_TXS), 8388607.5, 8388608.0,
                        Alu.add, Alu.subtract)
        v.tensor_tensor(s(S_WX), s(S_TXS), s(S_X0S), op=Alu.subtract)
        v.tensor_scalar(s(S_X1S), s(S_X0S), 1.0, None, Alu.add)
        v.tensor_scalar(s(S_XB), s(S_X0S), 16.0, None, Alu.max)
        v.tensor_scalar(s(S_XB), s(S_XB), 142.0, None, Alu.min)
        v.tensor_scalar(s(S_XB1), s(S_XB), 1.0, None, Alu.add)
        # slot weights: as_m = (1-wx)*[x0==xb+m] + wx*[x1==xb+m]
        v.tensor_scalar(s(S_AWX), s(S_WX), -1.0, 1.0, Alu.mult, Alu.add)
        v.tensor_tensor(s(S_T0), s(S_X0S), s(S_XB), op=Alu.is_equal)
        v.tensor_tensor(s(S_T1), s(S_X1S), s(S_XB), op=Alu.is_equal)
        v.tensor_tensor(s(S_T0), s(S_AWX), s(S_T0), op=Alu.mult)
        v.tensor_tensor(s(S_T1), s(S_WX), s(S_T1), op=Alu.mult)
        v.tensor_tensor(s(S_AS0), s(S_T0), s(S_T1), op=Alu.add)
        v.tensor_tensor(s(S_T0), s(S_X0S), s(S_XB1), op=Alu.is_equal)
        v.tensor_tensor(s(S_T1), s(S_X1S), s(S_XB1), op=Alu.is_equal)
        v.tensor_tensor(s(S_T0), s(S_AWX), s(S_T0), op=Alu.mult)
        v.tensor_tensor(s(S_T1), s(S_WX), s(S_T1), op=Alu.mult)
        v.tensor_tensor(s(S_AS1), s(S_T0), s(S_T1), op=Alu.add)
        # y weights with validity and mask folded in
        v.tensor_scalar(s(S_AWY), s(S_WY), -1.0, 1.0, Alu.mult, Alu.add)
        v.tensor_tensor(s(S_WY0M), s(S_AWY), s(S_V0), op=Alu.mult)
        v.tensor_tensor(s(S_WY0M), s(S_WY0M), s(S_MSK), op=Alu.mult)
        v.tensor_tensor(s(S_WY1M), s(S_WY), s(S_V1), op=Alu.mult)
        v.tensor_tensor(s(S_WY1M), s(S_WY1M), s(S_MSK), op=Alu.mult)
        v.tensor_tensor(W4[:, 0], s(S_WY0M), s(S_AS0), op=Alu.mult)
        v.tensor_tensor(W4[:, 1], s(S_WY0M), s(S_AS1), op=Alu.mult)
        v.tensor_tensor(W4[:, 2], s(S_WY1M), s(S_AS0), op=Alu.mult)
        v.tensor_tensor(W4[:, 3], s(S_WY1M), s(S_AS1), op=Alu.mult)
        # gather unit index = y0c*128 + xb + (128*(h-16-ylo) - 16)
        v.tensor_tensor(s(S_ADR), s(S_XB), HOFb, op=Alu.add)
        v.scalar_tensor_tensor(s(S_I0F), s(S_Y0C), 128.0, s(S_ADR),
                               Alu.mult, Alu.add)
        v.scalar_tensor_tensor(s(S_I1F), s(S_Y1C), 128.0, s(S_ADR),
                               Alu.mult, Alu.add)
        i0v = s(S_I0F).rearrange("p (g r) k -> p g r k", r=GG)
        i1v = s(S_I1F).rearrange("p (g r) k -> p g r k", r=GG)
        v.tensor_copy(IAL[:, :, :, 0, :].rearrange("p g k r -> p g r k"), i0v)
        v.tensor_copy(IAL[:, :, :, 1, :].rearrange("p g k r -> p g r k"), i1v)

        # ---- wrap indices into the 16-partition gather layout + replicate ----
        for sw in range(8):
            src = IAL[16 * sw:16 * (sw + 1)].rearrange("p g k c r -> p (g k c r)")
            nc.sync.dma_start(WR[0:16, :, :, :, :, sw], src)
        nc.sync.dma_start(WR[16:32], WR[0:16])
        nc.sync.dma_start(WR[32:64], WR[0:32])
        nc.sync.dma_start(WR[64:128], WR[0:64])

        # ---- merged gather + wk scale + MAC + pointwise per group ----
        for gg in range(NG):
            GT = gpool.tile([128, K2 * 2 * GG, 256], DT.float16, tag="gt",
                            name="gt")
            src = bass.AP(tensor=xs, offset=0, ap=[[128, SLAB_U], [1, 256]])
            idxs = WR[:, gg].rearrange("p k c r s -> p (k c r s)")
            nc.gpsimd.dma_gather(GT[:], src, idxs, NIDX, nidx_reg, 256,
                                 elem_step=128, queue_num=gidx[0] % 4)
            gidx[0] += 1
            # apply depthwise weight wk[c,k] (2x-rate DVE tensor_tensor)
            for k in range(K2):
                wkv = WKR[:, k, :]
                wkb = bass.AP(tensor=wkv.tensor, offset=wkv.offset,
                              ap=[list(wkv.ap[0]), [0, 2 * GG], [1, 256]])
                gv = GT[:, k * 2 * GG:(k + 1) * 2 * GG, :]
                v.tensor_tensor(gv, gv, wkb, op=Alu.mult)

            TR4 = tr_ps.tile([128, GG, 128], DT.float32, tag="tr", name="tr")
            for rr in range(GG):
                rb = gg * GG + rr

                def gslice(k, c2, s2):
                    return GT[:, (k * 2 + c2) * GG + rr,
                              s2 * 128:(s2 + 1) * 128]

                ACCd = apool.tile([128, 128], DT.float16, tag="accd")
                first = True
                for (k, c2, s2) in dve_terms:
                    g = gslice(k, c2, s2)
                    wsc = W4[:, c2 * 2 + s2, rb, k:k + 1]
                    if first:
                        v.tensor_scalar(ACCd[:], g, wsc, None, Alu.mult)
                        first = False
                    else:
                        v.scalar_tensor_tensor(ACCd[:], g, wsc, ACCd[:],
                                               Alu.mult, Alu.add)
                ACCp = apool.tile([128, 128], DT.float16, tag="accp")
                first = True
                for (k, c2, s2) in pool_terms:
                    g = gslice(k, c2, s2)
                    wsc = W4[:, c2 * 2 + s2, rb, k:k + 1]
                    if first:
                        nc.gpsimd.tensor_scalar(ACCp[:], g, wsc, None,
                                                Alu.mult)
                        first = False
                    else:
                        nc.gpsimd.scalar_tensor_tensor(ACCp[:], g, wsc,
                                                       ACCp[:], Alu.mult,
                                                       Alu.add)
                # transpose both accumulators into [c, w], summing in PSUM
                nc.tensor.matmul(TR4[:, rr, :], ACCd[:], IDN[:],
                                 start=True, stop=False)
                nc.tensor.matmul(TR4[:, rr, :], ACCp[:], IDN[:],
                                 start=False, stop=True)

            RT4 = rpool.tile([128, GG, 128], DT.float16, tag="rt4")
            nc.scalar.activation(RT4[:], TR4[:], AF.Copy)
            OUTS = opool.tile([128, 2, GG, 128], DT.float16, tag="outs")
            PW = pw_ps.tile([128, 2, GG, 128], DT.float32, tag="pw",
                            name="pw")
            rt_flat = RT4[:].rearrange("p g w -> p (g w)")
            for oh in range(2):
                nc.tensor.matmul(PW[:, oh], WPW[:, oh * 128:(oh + 1) * 128],
                                 rt_flat, start=True, stop=True)
                nc.scalar.activation(OUTS[:, oh], PW[:, oh], AF.Identity,
                                     bias=BPW[:, oh:oh + 1])
            r0 = bt * RB + gg * GG
            nc.sync.dma_start(out_v[:, :, r0:r0 + GG, :], OUTS[:])


# ---------------- host side ----------------

def host_prep(inputs):
    x = np.asarray(inputs["x"], np.float32)
    w_off = np.asarray(inputs["w_off"], np.float32)
    b_off = np.asarray(inputs["b_off"], np.float32)
    w_dw = np.asarray(inputs["w_dw"], np.float32)
    b_dw = np.asarray(inputs["b_dw"], np.float32)
    w_pw = np.asarray(inputs["w_pw"], np.float32)
    b_pw = np.asarray(inputs["b_pw"], np.float32)

    wk = w_dw.reshape(C, K2)
    woff_p = np.ascontiguousarray(
        w_off.transpose(1, 2, 3, 0).reshape(C, K2 * 27)).astype(np.float16)
    wpw_p = np.ascontiguousarray(w_pw.T).astype(np.float16)
    idn = np.eye(128, dtype=np.float16)
    bpw_eff = (b_pw + w_pw @ b_dw).astype(np.float32)

    # wk replicated across partitions, per (k, slot, c)
    wkrow = np.tile(wk.T[:, None, :], (1, 2, 1)).reshape(-1)  # [K2*256]
    wkr = np.ascontiguousarray(
        np.broadcast_to(wkrow, (128, K2 * 256))).astype(np.float16)

    ky = (np.arange(K2) // 3 - 1).astype(np.float32)
    kx = (np.arange(K2) % 3 - 1).astype(np.float32)

    # pixel-major fp16 image per batch
    xts = [np.ascontiguousarray(x[b].transpose(1, 2, 0)).astype(np.float16)
           .reshape(H * W, C) for b in range(B)]

    in_maps = []
    for core in range(8):
        b, half = core // 2, core % 2
        r0 = half * ROWS
        ylo = 0 if half == 0 else H - SLAB_ROWS
        xcp = np.zeros([C, 66, 130], np.float32)
        lo, hi = max(r0 - 1, 0), min(r0 + 65, H)
        xcp[:, lo - (r0 - 1): hi - (r0 - 1), 1:129] = x[b][:, lo:hi, :]
        xcp = xcp.astype(np.float16).reshape(C, 66 * 130)

        xsp = np.zeros([SLAB_U + 1, 128], np.float16)
        xsp[0:SLAB_U] = xts[b][ylo * 128:(ylo + SLAB_ROWS) * 128]

        hvec = (r0 + np.arange(ROWS)).astype(np.float32)
        cons = np.zeros([128, CONS_W], np.float32)
        cons[:, 0:9] = ky[None, :] + 16.0
        cons[:, 9:18] = kx[None, :] + 16.0 + np.arange(128, dtype=np.float32)[:, None]
        cons[:, 18:82] = (ylo + 16.0 - hvec)[None, :]
        cons[:, 82:146] = (min(143.0, ylo + SLAB_ROWS + 15.0) - hvec)[None, :]
        cons[:, 146:210] = (128.0 * (hvec - 16.0 - ylo) - 16.0)[None, :]
        cons[:, 210:212] = bpw_eff.reshape(2, 128).T
        cons[:, 212:239] = b_off[None, :]

        in_maps.append({
            "xc": xcp, "xs": xsp, "woff": woff_p, "wpw": wpw_p,
            "idn": idn, "wkr": wkr, "cons": cons,
        })
    return in_maps


def assemble(results):
    out = np.zeros([B, O, H, W], np.float32)
    for core, r in enumerate(results):
        b, half = core // 2, core % 2
        out[b, :, half * ROWS:(half + 1) * ROWS, :] = \
            r["out"].astype(np.float32)
    return out


# ---- single-sync-wait legalization (inlined) ----
_doc = """Legalize BIR for walrus builds that allow only ONE sync wait per
instruction: hoist extra waits onto same-engine NOPs inserted immediately
before the offending instruction."""
import copy

def _make_nop(nc, engine):
    nop = nc.engines[engine].nop(nofuse=True).ins
    # the builder appended it to nc.cur_bb; steal it from wherever it landed
    for f in nc.m.functions:
        for bb in f.blocks:
            il = bb.instructions
            if il and il[-1].name == nop.name:
                il.pop()
                bb.instructions = il
                return nop
    raise RuntimeError("freshly built nop not found")

def legalize_single_wait(nc):
    n_split = 0
    for f in nc.m.functions:
        for bb in f.blocks:
            insts = bb.instructions
            if not any(i.sync_info and len(i.sync_info.on_wait) > 1 for i in insts):
                continue
            out = []
            for inst in insts:
                si = inst.sync_info
                if si and len(si.on_wait) > 1:
                    waits = list(si.on_wait)
                    for w in waits[:-1]:
                        nop = _make_nop(nc, inst.engine)
                        nsi = copy.deepcopy(si)
                        nsi.on_wait = [w]
                        nsi.on_update = []
                        nop.sync_info = nsi
                        out.append(nop)
                    si.on_wait = [waits[-1]]
                    n_split += 1
                out.append(inst)
            bb.instructions = out
    return n_split


_CACHED_NC = None


def kernel(**inputs):
    global _CACHED_NC
    from concourse import bass_utils
    in_maps = host_prep(inputs)
    if _CACHED_NC is None:
        _CACHED_NC = build_nc()
    res = bass_utils.run_bass_kernel_spmd(_CACHED_NC, in_maps,
                                          core_ids=list(range(8)))
    return assemble(res.results)


# revision 10
# speedup vs baseline: 2.3996x; 2.3996x over previous
"""DepthwiseSeparableDCNv2 for Trainium2 — self-contained 8-core SPMD Bass kernel.

kernel(**inputs) takes the full unsharded inputs and returns the full
[4, 256, 128, 128] float32 output. Sharding: 4 batch samples x 2 H-halves.

v2 pipeline per core (vs the v1 baseline):
  - gathers read a single unscaled pixel-major slab (80 rows + halo) from
    DRAM; the per-tap depthwise weight wk[c,k] is applied on-device with a
    2x-rate tensor_tensor against a partition-replicated wk tile, instead
    of shipping a 9-tap pre-scaled 37.7 MB image from the host.
  - one dma_gather per 4-row group covers all 9 taps (9216 indices).
  - the 36-term bilinear MAC per row is split between the DVE and Pool
    engines into two accumulators, which the PE transpose sums for free
    via PSUM accumulation.
  - pointwise conv + bias run batched over 4 rows; output is fp16.
"""
import numpy as np
import ml_dtypes
from contextlib import ExitStack

import concourse.bass as bass
from concourse import bacc
import concourse.mybir as mybir
from concourse.tile import TileContext
from concourse._compat import with_exitstack
from concourse import library_config

DT = mybir.dt
Alu = mybir.AluOpType
AF = mybir.ActivationFunctionType

B, C, H, W, O = 4, 128, 128, 128, 256
K2 = 9
ROWS = 64          # output rows per core
RB = 32            # idx-math batch rows
GG = 4             # rows per gather group
NG = RB // GG      # gather groups per batch
NIDX = GG * 2 * K2 * 128   # indices per merged gather instruction (9216)
SLAB_ROWS = 80     # slab rows held per core (64 out rows + 16 halo)
SLAB_U = SLAB_ROWS * 128   # slab units
N_DVE = 10         # MAC terms per row on DVE; remaining 36-N_DVE on Pool

CONS_W = 9 + 9 + 64 + 64 + 64 + 2 + 27  # 239

# scratch slot ids in the consolidated [128, NS, RB, 9] f32 tile
(S_MSK, S_WY, S_Y0S, S_Y1S, S_V0, S_V1, S_Y0C, S_Y1C, S_WX, S_X0S, S_X1S,
 S_XB, S_XB1, S_AS0, S_AS1, S_T0, S_T1, S_AWX, S_AWY, S_WY0M, S_WY1M,
 S_TMP) = range(22)
NS = 22
S_TYS = S_TMP   # tys -> txs -> adr share one slot (sequential lifetimes)
S_TXS = S_TMP
S_ADR = S_TMP
S_I0F = S_V0    # v0/v1 dead once wy0m/wy1m built
S_I1F = S_V1


def build_nc():
    nc = bacc.Bacc("TRN2", target_bir_lowering=False, debug=False,
                   num_devices=8, num_swdge_queues=4)
    xc = nc.dram_tensor("xc", [128, 66 * 130], DT.float16, kind="ExternalInput")
    xs = nc.dram_tensor("xs", [SLAB_U + 1, 128], DT.float16, kind="ExternalInput")
    woff = nc.dram_tensor("woff", [128, K2 * 27], DT.float16, kind="ExternalInput")
    wpw = nc.dram_tensor("wpw", [128, 256], DT.float16, kind="ExternalInput")
    idn = nc.dram_tensor("idn", [128, 128], DT.float16, kind="ExternalInput")
    wkr = nc.dram_tensor("wkr", [128, K2 * 256], DT.float16, kind="ExternalInput")
    cons = nc.dram_tensor("cons", [128, CONS_W], DT.float32, kind="ExternalInput")
    out = nc.dram_tensor("out", [256, ROWS, 128], DT.float16, kind="ExternalOutput")

    with TileContext(nc) as tc:
        _kernel(tc, xc, xs, woff, wpw, idn, wkr, cons, out)

    nc.compile()
    legalize_single_wait(nc)
    bass.Bass.finalize(nc)
    return nc


@with_exitstack
def _kernel(ctx: ExitStack, tc: TileContext, xc, xs, woff, wpw, idn, wkr,
            cons, out):
    nc = tc.nc

    cpool = ctx.enter_context(tc.tile_pool(name="const", bufs=1))
    XC = cpool.tile([128, 66 * 130], DT.float16)
    nc.sync.dma_start(XC[:], xc.ap())
    WOF = cpool.tile([128, K2, 27], DT.float16)
    nc.sync.dma_start(WOF[:], woff.ap())
    WPW = cpool.tile([128, 256], DT.float16)
    nc.sync.dma_start(WPW[:], wpw.ap())
    IDN = cpool.tile([128, 128], DT.float16)
    nc.sync.dma_start(IDN[:], idn.ap())
    WKR = cpool.tile([128, K2, 256], DT.float16)
    nc.sync.dma_start(WKR[:], wkr.ap())
    CON = cpool.tile([128, CONS_W], DT.float32)
    nc.sync.dma_start(CON[:], cons.ap())

    KY = CON[:, 0:9]           # ky + 16                  [128, 9]
    KX = CON[:, 9:18]          # w + kx + 16              [128, 9]
    HL = CON[:, 18:82]         # slab lo clamp per row    [128, 64]
    HH = CON[:, 82:146]        # slab hi clamp per row    [128, 64]
    HOF = CON[:, 146:210]      # slab unit offset per row [128, 64]
    BPW = CON[:, 210:212]      # fused pointwise bias     [128, 2]

    om_ps = ctx.enter_context(tc.tile_pool(name="omp", bufs=2, space="PSUM"))
    tr_ps = ctx.enter_context(tc.tile_pool(name="trp", bufs=2, space="PSUM"))
    pw_ps = ctx.enter_context(tc.tile_pool(name="pwp", bufs=2, space="PSUM"))
    oms_pool = ctx.enter_context(tc.tile_pool(name="oms", bufs=2))
    mpool = ctx.enter_context(tc.tile_pool(name="m", bufs=1))
    wpool = ctx.enter_context(tc.tile_pool(name="wp", bufs=2))
    ipool = ctx.enter_context(tc.tile_pool(name="ip", bufs=2))
    wrpool = ctx.enter_context(tc.tile_pool(name="wr", bufs=2))
    gpool = ctx.enter_context(tc.tile_pool(name="g", bufs=2))
    apool = ctx.enter_context(tc.tile_pool(name="acc", bufs=2))
    rpool = ctx.enter_context(tc.tile_pool(name="rt", bufs=2))
    opool = ctx.enter_context(tc.tile_pool(name="o", bufs=2))

    out_v = out.ap().rearrange("(oh o) r w -> o oh r w", oh=2)
    gidx = [0]
    nidx_regs = {}
    for nsl in (96,):
        reg = ctx.enter_context(nc.gpsimd.register(f"nidx{nsl}"))
        nc.gpsimd.reg_mov(reg, nsl * 16)
        nidx_regs[nsl] = reg

    # all 36 MAC terms per row run on DVE (Pool can't execute
    # TensorScalarPtr, and Pool tensor_tensor needs a different GPSIMD
    # library than dma_gather)
    dve_terms = [(k, c2, s2) for k in range(K2) for c2 in range(2)
                 for s2 in range(2)]

    for bt in range(2):
        # ---- offset conv: om.T [w, 27] per row ----
        OMS = oms_pool.tile([128, RB, 27], DT.float32, tag="oms")
        for r in range(RB):
            om = om_ps.tile([128, 27], DT.float32, tag="om", name="om")
            pos = (bt * RB + r + 1) * 130 + 1
            for t in range(K2):
                ty, tx = t // 3, t % 3
                sh = (ty - 1) * 130 + (tx - 1)
                nc.tensor.matmul(om[:], XC[:, pos + sh: pos + sh + 128],
                                 WOF[:, t, :], start=(t == 0), stop=(t == 8))
            nc.scalar.activation(OMS[:, r, :], om[:], AF.Copy)
        # b_off (broadcast over rows)
        _bof = CON[:, 212:239]
        bof_b = bass.AP(tensor=_bof.tensor, offset=_bof.offset,
                        ap=[list(_bof.ap[0]), [0, RB], [1, 27]])
        nc.vector.tensor_tensor(OMS[:], OMS[:], bof_b, op=Alu.add)

        # ---- index / weight math ----
        SCR = mpool.tile([128, NS, RB, K2], DT.float32, tag="scr", name="scr")

        def s(i):
            return SCR[:, i]

        nc.scalar.activation(s(S_MSK), OMS[:, :, 18:27], AF.Sigmoid)

        offs = OMS[:, :, 0:18].rearrange("p r (k two) -> p two r k", two=2)
        dy, dx = offs[:, 0], offs[:, 1]

        def bc9(ap128x9):   # [128, 9] -> [128, RB, 9] broadcast over rows
            return bass.AP(tensor=ap128x9.tensor, offset=ap128x9.offset,
                           ap=[list(ap128x9.ap[0]), [0, RB], [1, 9]])

        def bcrow(ap128x64):  # [128, 64] row-consts -> [128, RB, 9] for batch bt
            sl = ap128x64[:, bt * RB:(bt + 1) * RB]
            return bass.AP(tensor=sl.tensor, offset=sl.offset,
                           ap=[list(sl.ap[0]), [1, RB], [0, 9]])

        KYb, KXb = bc9(KY), bc9(KX)
        HLb, HHb, HOFb = bcrow(HL), bcrow(HH), bcrow(HOF)
        v = nc.vector

        W4 = wpool.tile([128, 4, RB, K2], DT.float32, tag="w4")
        IAL = ipool.tile([128, NG, K2, 2, GG], DT.int16, tag="ial")
        WR = wrpool.tile([128, NG, K2, 2, GG, 8], DT.int16, tag="wr")

        v.tensor_tensor(s(S_TYS), dy, KYb, op=Alu.add)
        v.tensor_scalar(s(S_TYS), s(S_TYS), 0.0, None, Alu.max)
        # floor via the 2^23 magic number: RNE(x - 0.5) == floor(x) up to
        # integer ties, which bilinear continuity makes harmless
        v.tensor_scalar(s(S_Y0S), s(S_TYS), 8388607.5, 8388608.0,
                        Alu.add, Alu.subtract)
        v.tensor_tensor(s(S_WY), s(S_TYS), s(S_Y0S), op=Alu.subtract)
        v.tensor_scalar(s(S_Y1S), s(S_Y0S), 1.0, None, Alu.add)
        v.tensor_tensor(s(S_T0), s(S_Y0S), HLb, op=Alu.is_ge)
        v.tensor_tensor(s(S_T1), s(S_Y0S), HHb, op=Alu.is_le)
        v.tensor_tensor(s(S_V0), s(S_T0), s(S_T1), op=Alu.mult)
        v.tensor_tensor(s(S_T0), s(S_Y1S), HLb, op=Alu.is_ge)
        v.tensor_tensor(s(S_T1), s(S_Y1S), HHb, op=Alu.is_le)
        v.tensor_tensor(s(S_V1), s(S_T0), s(S_T1), op=Alu.mult)
        v.tensor_tensor(s(S_Y0C), s(S_Y0S), HLb, op=Alu.max)
        v.tensor_tensor(s(S_Y0C), s(S_Y0C), HHb, op=Alu.min)
        v.tensor_tensor(s(S_Y1C), s(S_Y1S), HLb, op=Alu.max)
        v.tensor_tensor(s(S_Y1C), s(S_Y1C), HHb, op=Alu.min)

        v.tensor_tensor(s(S_TXS), dx, KXb, op=Alu.add)
        v.tensor_scalar(s(S_TXS), s(S_TXS), 0.0, None, Alu.max)
        v.tensor_scalar(s(S_X0S), s(S_TXS), 8388607.5, 8388608.0,
                        Alu.add, Alu.subtract)
        v.tensor_tensor(s(S_WX), s(S_TXS), s(S_X0S), op=Alu.subtract)
        v.tensor_scalar(s(S_X1S), s(S_X0S), 1.0, None, Alu.add)
        v.tensor_scalar(s(S_XB), s(S_X0S), 16.0, None, Alu.max)
        v.tensor_scalar(s(S_XB), s(S_XB), 142.0, None, Alu.min)
        v.tensor_scalar(s(S_XB1), s(S_XB), 1.0, None, Alu.add)
        # slot weights: as_m = (1-wx)*[x0==xb+m] + wx*[x1==xb+m]
        v.tensor_scalar(s(S_AWX), s(S_WX), -1.0, 1.0, Alu.mult, Alu.add)
        v.tensor_tensor(s(S_T0), s(S_X0S), s(S_XB), op=Alu.is_equal)
        v.tensor_tensor(s(S_T1), s(S_X1S), s(S_XB), op=Alu.is_equal)
        v.tensor_tensor(s(S_T0), s(S_AWX), s(S_T0), op=Alu.mult)
        v.tensor_tensor(s(S_T1), s(S_WX), s(S_T1), op=Alu.mult)
        v.tensor_tensor(s(S_AS0), s(S_T0), s(S_T1), op=Alu.add)
        v.tensor_tensor(s(S_T0), s(S_X0S), s(S_XB1), op=Alu.is_equal)
        v.tensor_tensor(s(S_T1), s(S_X1S), s(S_XB1), op=Alu.is_equal)
        v.tensor_tensor(s(S_T0), s(S_AWX), s(S_T0), op=Alu.mult)
        v.tensor_tensor(s(S_T1), s(S_WX), s(S_T1), op=Alu.mult)
        v.tensor_tensor(s(S_AS1), s(S_T0), s(S_T1), op=Alu.add)
        # y weights with validity and mask folded in
        v.tensor_scalar(s(S_AWY), s(S_WY), -1.0, 1.0, Alu.mult, Alu.add)
        v.tensor_tensor(s(S_WY0M), s(S_AWY), s(S_V0), op=Alu.mult)
        v.tensor_tensor(s(S_WY0M), s(S_WY0M), s(S_MSK), op=Alu.mult)
        v.tensor_tensor(s(S_WY1M), s(S_WY), s(S_V1), op=Alu.mult)
        v.tensor_tensor(s(S_WY1M), s(S_WY1M), s(S_MSK), op=Alu.mult)
        v.tensor_tensor(W4[:, 0], s(S_WY0M), s(S_AS0), op=Alu.mult)
        v.tensor_tensor(W4[:, 1], s(S_WY0M), s(S_AS1), op=Alu.mult)
        v.tensor_tensor(W4[:, 2], s(S_WY1M), s(S_AS0), op=Alu.mult)
        v.tensor_tensor(W4[:, 3], s(S_WY1M), s(S_AS1), op=Alu.mult)
        # gather unit index = y0c*128 + xb + (128*(h-16-ylo) - 16)
        v.tensor_tensor(s(S_ADR), s(S_XB), HOFb, op=Alu.add)
        v.scalar_tensor_tensor(s(S_I0F), s(S_Y0C), 128.0, s(S_ADR),
                               Alu.mult, Alu.add)
        v.scalar_tensor_tensor(s(S_I1F), s(S_Y1C), 128.0, s(S_ADR),
                               Alu.mult, Alu.add)
        i0v = s(S_I0F).rearrange("p (g r) k -> p g r k", r=GG)
        i1v = s(S_I1F).rearrange("p (g r) k -> p g r k", r=GG)
        v.tensor_copy(IAL[:, :, :, 0, :].rearrange("p g k r -> p g r k"), i0v)
        v.tensor_copy(IAL[:, :, :, 1, :].rearrange("p g k r -> p g r k"), i1v)

        # ---- wrap indices into the 16-partition gather layout + replicate ----
        for sw in range(8):
            src = IAL[16 * sw:16 * (sw + 1)].rearrange("p g k c r -> p (g k c r)")
            nc.sync.dma_start(WR[0:16, :, :, :, :, sw], src)
        nc.sync.dma_start(WR[16:32], WR[0:16])
        nc.sync.dma_start(WR[32:64], WR[0:32])
        nc.sync.dma_start(WR[64:128], WR[0:64])

        # ---- gather + wk scale + MAC + pointwise per group ----
        # the SWDGE descriptor ring holds 128 entries and each gather needs
        # num_idxs/16 + 1, so split each group's 9216 indices into 5 chunks
        for gg in range(NG):
            GT = gpool.tile([128, K2 * 2 * GG, 256], DT.float16, tag="gt",
                            name="gt")
            src = bass.AP(tensor=xs, offset=0, ap=[[128, SLAB_U], [1, 256]])
            idxs = WR[:, gg].rearrange("p k c r s -> p (k c r s)")
            for (sl0, nsl) in ((0, 96), (96, 96), (192, 96), (288, 96),
                               (384, 96), (480, 96)):
                nc.gpsimd.dma_gather(GT[:, sl0 // 8:(sl0 + nsl) // 8, :],
                                     src, idxs[:, sl0:sl0 + nsl],
                                     nsl * 16, nidx_regs[nsl], 256,
                                     elem_step=128, queue_num=gidx[0] % 4)
                gidx[0] += 1
            # apply depthwise weight wk[c,k] (2x-rate DVE tensor_tensor)
            for k in range(K2):
                wkv = WKR[:, k, :]
                wkb = bass.AP(tensor=wkv.tensor, offset=wkv.offset,
                              ap=[list(wkv.ap[0]), [0, 2 * GG], [1, 256]])
                gv = GT[:, k * 2 * GG:(k + 1) * 2 * GG, :]
                v.tensor_tensor(gv, gv, wkb, op=Alu.mult)

            TR4 = tr_ps.tile([128, GG, 128], DT.float32, tag="tr", name="tr")
            for rr in range(GG):
                rb = gg * GG + rr

                def gslice(k, c2, s2):
                    return GT[:, (k * 2 + c2) * GG + rr,
                              s2 * 128:(s2 + 1) * 128]

                ACCd = apool.tile([128, 128], DT.float16, tag="accd")
                first = True
                for (k, c2, s2) in dve_terms:
                    g = gslice(k, c2, s2)
                    wsc = W4[:, c2 * 2 + s2, rb, k:k + 1]
                    if first:
                        v.tensor_scalar(ACCd[:], g, wsc, None, Alu.mult)
                        first = False
                    else:
                        v.scalar_tensor_tensor(ACCd[:], g, wsc, ACCd[:],
                                               Alu.mult, Alu.add)
                # transpose the accumulator into [c, w]
                nc.tensor.matmul(TR4[:, rr, :], ACCd[:], IDN[:],
                                 start=True, stop=True)

            RT4 = rpool.tile([128, GG, 128], DT.float16, tag="rt4")
            nc.scalar.activation(RT4[:], TR4[:], AF.Copy)
            OUTS = opool.tile([128, 2, GG, 128], DT.float16, tag="outs")
            PW = pw_ps.tile([128, 2, GG, 128], DT.float32, tag="pw",
                            name="pw")
            rt_flat = RT4[:].rearrange("p g w -> p (g w)")
            for oh in range(2):
                nc.tensor.matmul(PW[:, oh], WPW[:, oh * 128:(oh + 1) * 128],
                                 rt_flat, start=True, stop=True)
                nc.scalar.activation(OUTS[:, oh], PW[:, oh], AF.Identity,
                                     bias=BPW[:, oh:oh + 1])
            r0 = bt * RB + gg * GG
            nc.sync.dma_start(out_v[:, :, r0:r0 + GG, :], OUTS[:])


# ---------------- host side ----------------

def host_prep(inputs):
    x = np.asarray(inputs["x"], np.float32)
    w_off = np.asarray(inputs["w_off"], np.float32)
    b_off = np.asarray(inputs["b_off"], np.float32)
    w_dw = np.asarray(inputs["w_dw"], np.float32)
    b_dw = np.asarray(inputs["b_dw"], np.float32)
    w_pw = np.asarray(inputs["w_pw"], np.float32)
    b_pw = np.asarray(inputs["b_pw"], np.float32)

    wk = w_dw.reshape(C, K2)
    woff_p = np.ascontiguousarray(
        w_off.transpose(1, 2, 3, 0).reshape(C, K2 * 27)).astype(np.float16)
    wpw_p = np.ascontiguousarray(w_pw.T).astype(np.float16)
    idn = np.eye(128, dtype=np.float16)
    bpw_eff = (b_pw + w_pw @ b_dw).astype(np.float32)

    # wk replicated across partitions, per (k, slot, c)
    wkrow = np.tile(wk.T[:, None, :], (1, 2, 1)).reshape(-1)  # [K2*256]
    wkr = np.ascontiguousarray(
        np.broadcast_to(wkrow, (128, K2 * 256))).astype(np.float16)

    ky = (np.arange(K2) // 3 - 1).astype(np.float32)
    kx = (np.arange(K2) % 3 - 1).astype(np.float32)

    # pixel-major fp16 image per batch
    xts = [np.ascontiguousarray(x[b].transpose(1, 2, 0)).astype(np.float16)
           .reshape(H * W, C) for b in range(B)]

    in_maps = []
    for core in range(8):
        b, half = core // 2, core % 2
        r0 = half * ROWS
        ylo = 0 if half == 0 else H - SLAB_ROWS
        xcp = np.zeros([C, 66, 130], np.float32)
        lo, hi = max(r0 - 1, 0), min(r0 + 65, H)
        xcp[:, lo - (r0 - 1): hi - (r0 - 1), 1:129] = x[b][:, lo:hi, :]
        xcp = xcp.astype(np.float16).reshape(C, 66 * 130)

        xsp = np.zeros([SLAB_U + 1, 128], np.float16)
        xsp[0:SLAB_U] = xts[b][ylo * 128:(ylo + SLAB_ROWS) * 128]

        hvec = (r0 + np.arange(ROWS)).astype(np.float32)
        cons = np.zeros([128, CONS_W], np.float32)
        cons[:, 0:9] = ky[None, :] + 16.0
        cons[:, 9:18] = kx[None, :] + 16.0 + np.arange(128, dtype=np.float32)[:, None]
        cons[:, 18:82] = (ylo + 16.0 - hvec)[None, :]
        cons[:, 82:146] = (min(143.0, ylo + SLAB_ROWS + 15.0) - hvec)[None, :]
        cons[:, 146:210] = (128.0 * (hvec - 16.0 - ylo) - 16.0)[None, :]
        cons[:, 210:212] = bpw_eff.reshape(2, 128).T
        cons[:, 212:239] = b_off[None, :]

        in_maps.append({
            "xc": xcp, "xs": xsp, "woff": woff_p, "wpw": wpw_p,
            "idn": idn, "wkr": wkr, "cons": cons,
        })
    return in_maps


def assemble(results):
    out = np.zeros([B, O, H, W], np.float32)
    for core, r in enumerate(results):
        b, half = core // 2, core % 2
        out[b, :, half * ROWS:(half + 1) * ROWS, :] = \
            r["out"].astype(np.float32)
    return out


# ---- single-sync-wait legalization (inlined) ----
_doc = """Legalize BIR for walrus builds that allow only ONE sync wait per
instruction: hoist extra waits onto same-engine NOPs inserted immediately
before the offending instruction."""
import copy

def _make_nop(nc, engine):
    nop = nc.engines[engine].nop(nofuse=True).ins
    # the builder appended it to nc.cur_bb; steal it from wherever it landed
    for f in nc.m.functions:
        for bb in f.blocks:
            il = bb.instructions
            if il and il[-1].name == nop.name:
                il.pop()
                bb.instructions = il
                return nop
    raise RuntimeError("freshly built nop not found")

def legalize_single_wait(nc):
    n_split = 0
    for f in nc.m.functions:
        for bb in f.blocks:
            insts = bb.instructions
            if not any(i.sync_info and len(i.sync_info.on_wait) > 1 for i in insts):
                continue
            out = []
            for inst in insts:
                si = inst.sync_info
                if si and len(si.on_wait) > 1:
                    waits = list(si.on_wait)
                    for w in waits[:-1]:
                        nop = _make_nop(nc, inst.engine)
                        nsi = copy.deepcopy(si)
                        nsi.on_wait = [w]
                        nsi.on_update = []
                        nop.sync_info = nsi
                        out.append(nop)
                    si.on_wait = [waits[-1]]
                    n_split += 1
                out.append(inst)
            bb.instructions = out
    return n_split


_CACHED_NC = None


def kernel(**inputs):
    global _CACHED_NC
    from concourse import bass_utils
    in_maps = host_prep(inputs)
    if _CACHED_NC is None:
        _CACHED_NC = build_nc()
    res = bass_utils.run_bass_kernel_spmd(_CACHED_NC, in_maps,
                                          core_ids=list(range(8)))
    return assemble(res.results)


# revision 12
# speedup vs baseline: 3.4743x; 1.4479x over previous
"""DepthwiseSeparableDCNv2 for Trainium2 — self-contained 8-core SPMD Bass kernel.

kernel(**inputs) takes the full unsharded inputs and returns the full
[4, 256, 128, 128] float32 output. Sharding: 4 batch samples x 2 H-halves.

v2 pipeline per core (vs the v1 baseline):
  - gathers read a single unscaled pixel-major slab (80 rows + halo) from
    DRAM; the per-tap depthwise weight wk[c,k] is applied on-device with a
    2x-rate tensor_tensor against a partition-replicated wk tile, instead
    of shipping a 9-tap pre-scaled 37.7 MB image from the host.
  - one dma_gather per 4-row group covers all 9 taps (9216 indices).
  - the 36-term bilinear MAC per row is split between the DVE and Pool
    engines into two accumulators, which the PE transpose sums for free
    via PSUM accumulation.
  - pointwise conv + bias run batched over 4 rows; output is fp16.
"""
import numpy as np
import ml_dtypes
from contextlib import ExitStack

import concourse.bass as bass
from concourse import bacc
import concourse.mybir as mybir
from concourse.tile import TileContext
from concourse._compat import with_exitstack
from concourse import library_config

DT = mybir.dt
Alu = mybir.AluOpType
AF = mybir.ActivationFunctionType

B, C, H, W, O = 4, 128, 128, 128, 256
K2 = 9
ROWS = 64          # output rows per core
RB = 32            # idx-math batch rows
GG = 4             # rows per gather group
NG = RB // GG      # gather groups per batch
NIDX = GG * 2 * K2 * 128   # indices per merged gather instruction (9216)
SLAB_ROWS = 80     # slab rows held per core (64 out rows + 16 halo)
SLAB_U = SLAB_ROWS * 128   # slab units
N_DVE = 10         # MAC terms per row on DVE; remaining 36-N_DVE on Pool

CONS_W = 9 + 9 + 64 + 64 + 64 + 2 + 27  # 239

# scratch slot ids in the consolidated [128, NS, RB, 9] f32 tile
(S_MSK, S_WY, S_Y0S, S_Y1S, S_V0, S_V1, S_Y0C, S_Y1C, S_WX, S_X0S, S_X1S,
 S_XB, S_XB1, S_AS0, S_AS1, S_T0, S_T1, S_AWX, S_AWY, S_WY0M, S_WY1M,
 S_TMP) = range(22)
NS = 22
S_TYS = S_TMP   # tys -> txs -> adr share one slot (sequential lifetimes)
S_TXS = S_TMP
S_ADR = S_TMP
S_I0F = S_V0    # v0/v1 dead once wy0m/wy1m built
S_I1F = S_V1


def build_nc():
    nc = bacc.Bacc("TRN2", target_bir_lowering=False, debug=False,
                   num_devices=8, num_swdge_queues=4)
    xc = nc.dram_tensor("xc", [128, 66 * 130], DT.float16, kind="ExternalInput")
    xs = nc.dram_tensor("xs", [SLAB_U + 1, 128], DT.float16, kind="ExternalInput")
    woff = nc.dram_tensor("woff", [128, K2 * 27], DT.float16, kind="ExternalInput")
    wpw = nc.dram_tensor("wpw", [128, 256], DT.float16, kind="ExternalInput")
    idn = nc.dram_tensor("idn", [128, 128], DT.float16, kind="ExternalInput")
    wkr = nc.dram_tensor("wkr", [128, K2 * 256], DT.float16, kind="ExternalInput")
    cons = nc.dram_tensor("cons", [128, CONS_W], DT.float32, kind="ExternalInput")
    out = nc.dram_tensor("out", [256, ROWS, 128], DT.float16, kind="ExternalOutput")

    with TileContext(nc) as tc:
        _kernel(tc, xc, xs, woff, wpw, idn, wkr, cons, out)

    nc.compile()
    legalize_single_wait(nc)
    bass.Bass.finalize(nc)
    return nc


@with_exitstack
def _kernel(ctx: ExitStack, tc: TileContext, xc, xs, woff, wpw, idn, wkr,
            cons, out):
    nc = tc.nc

    cpool = ctx.enter_context(tc.tile_pool(name="const", bufs=1))
    XC = cpool.tile([128, 66 * 130], DT.float16)
    nc.sync.dma_start(XC[:], xc.ap())
    WOF = cpool.tile([128, K2, 27], DT.float16)
    nc.sync.dma_start(WOF[:], woff.ap())
    WPW = cpool.tile([128, 256], DT.float16)
    nc.sync.dma_start(WPW[:], wpw.ap())
    IDN = cpool.tile([128, 128], DT.float16)
    nc.sync.dma_start(IDN[:], idn.ap())
    WKR = cpool.tile([128, K2, 256], DT.float16)
    nc.sync.dma_start(WKR[:], wkr.ap())
    CON = cpool.tile([128, CONS_W], DT.float32)
    nc.sync.dma_start(CON[:], cons.ap())

    KY = CON[:, 0:9]           # ky + 16                  [128, 9]
    KX = CON[:, 9:18]          # w + kx + 16              [128, 9]
    HL = CON[:, 18:82]         # slab lo clamp per row    [128, 64]
    HH = CON[:, 82:146]        # slab hi clamp per row    [128, 64]
    HOF = CON[:, 146:210]      # slab unit offset per row [128, 64]
    BPW = CON[:, 210:212]      # fused pointwise bias     [128, 2]

    om_ps = ctx.enter_context(tc.tile_pool(name="omp", bufs=2, space="PSUM"))
    tr_ps = ctx.enter_context(tc.tile_pool(name="trp", bufs=2, space="PSUM"))
    pw_ps = ctx.enter_context(tc.tile_pool(name="pwp", bufs=2, space="PSUM"))
    oms_pool = ctx.enter_context(tc.tile_pool(name="oms", bufs=2))
    mpool = ctx.enter_context(tc.tile_pool(name="m", bufs=1))
    wpool = ctx.enter_context(tc.tile_pool(name="wp", bufs=2))
    ipool = ctx.enter_context(tc.tile_pool(name="ip", bufs=2))
    wrpool = ctx.enter_context(tc.tile_pool(name="wr", bufs=2))
    gpool = ctx.enter_context(tc.tile_pool(name="g", bufs=2))
    apool = ctx.enter_context(tc.tile_pool(name="acc", bufs=2))
    rpool = ctx.enter_context(tc.tile_pool(name="rt", bufs=2))
    opool = ctx.enter_context(tc.tile_pool(name="o", bufs=2))

    out_v = out.ap().rearrange("(oh o) r w -> o oh r w", oh=2)
    gidx = [0]
    nidx_regs = {}
    for nsl in (64,):
        reg = ctx.enter_context(nc.gpsimd.register(f"nidx{nsl}"))
        nc.gpsimd.reg_mov(reg, nsl * 16)
        nidx_regs[nsl] = reg

    # all 36 MAC terms per row run on DVE (Pool can't execute
    # TensorScalarPtr, and Pool tensor_tensor needs a different GPSIMD
    # library than dma_gather)
    dve_terms = [(k, c2, s2) for k in range(K2) for c2 in range(2)
                 for s2 in range(2)]

    for bt in range(2):
        # ---- offset conv: om.T [w, 27] per row ----
        OMS = oms_pool.tile([128, RB, 27], DT.float32, tag="oms")
        for r in range(RB):
            om = om_ps.tile([128, 27], DT.float32, tag="om", name="om")
            pos = (bt * RB + r + 1) * 130 + 1
            for t in range(K2):
                ty, tx = t // 3, t % 3
                sh = (ty - 1) * 130 + (tx - 1)
                nc.tensor.matmul(om[:], XC[:, pos + sh: pos + sh + 128],
                                 WOF[:, t, :], start=(t == 0), stop=(t == 8))
            nc.scalar.activation(OMS[:, r, :], om[:], AF.Copy)
        # b_off (broadcast over rows)
        _bof = CON[:, 212:239]
        bof_b = bass.AP(tensor=_bof.tensor, offset=_bof.offset,
                        ap=[list(_bof.ap[0]), [0, RB], [1, 27]])
        nc.vector.tensor_tensor(OMS[:], OMS[:], bof_b, op=Alu.add)

        # ---- index / weight math ----
        SCR = mpool.tile([128, NS, RB, K2], DT.float32, tag="scr", name="scr")

        def s(i):
            return SCR[:, i]

        nc.scalar.activation(s(S_MSK), OMS[:, :, 18:27], AF.Sigmoid)

        offs = OMS[:, :, 0:18].rearrange("p r (k two) -> p two r k", two=2)
        dy, dx = offs[:, 0], offs[:, 1]

        def bc9(ap128x9):   # [128, 9] -> [128, RB, 9] broadcast over rows
            return bass.AP(tensor=ap128x9.tensor, offset=ap128x9.offset,
                           ap=[list(ap128x9.ap[0]), [0, RB], [1, 9]])

        def bcrow(ap128x64):  # [128, 64] row-consts -> [128, RB, 9] for batch bt
            sl = ap128x64[:, bt * RB:(bt + 1) * RB]
            return bass.AP(tensor=sl.tensor, offset=sl.offset,
                           ap=[list(sl.ap[0]), [1, RB], [0, 9]])

        KYb, KXb = bc9(KY), bc9(KX)
        HLb, HHb, HOFb = bcrow(HL), bcrow(HH), bcrow(HOF)
        v = nc.vector

        W4 = wpool.tile([128, 4, RB, K2], DT.float32, tag="w4")
        IAL = ipool.tile([128, NG, K2, 2, GG], DT.int16, tag="ial")
        WR = wrpool.tile([128, NG, K2, 2, GG, 8], DT.int16, tag="wr")

        v.tensor_tensor(s(S_TYS), dy, KYb, op=Alu.add)
        v.tensor_scalar(s(S_TYS), s(S_TYS), 0.0, None, Alu.max)
        # floor via the 2^23 magic number: RNE(x - 0.5) == floor(x) up to
        # integer ties, which bilinear continuity makes harmless
        v.tensor_scalar(s(S_Y0S), s(S_TYS), 8388607.5, 8388608.0,
                        Alu.add, Alu.subtract)
        v.tensor_tensor(s(S_WY), s(S_TYS), s(S_Y0S), op=Alu.subtract)
        v.tensor_scalar(s(S_Y1S), s(S_Y0S), 1.0, None, Alu.add)
        v.tensor_tensor(s(S_T0), s(S_Y0S), HLb, op=Alu.is_ge)
        v.tensor_tensor(s(S_T1), s(S_Y0S), HHb, op=Alu.is_le)
        v.tensor_tensor(s(S_V0), s(S_T0), s(S_T1), op=Alu.mult)
        v.tensor_tensor(s(S_T0), s(S_Y1S), HLb, op=Alu.is_ge)
        v.tensor_tensor(s(S_T1), s(S_Y1S), HHb, op=Alu.is_le)
        v.tensor_tensor(s(S_V1), s(S_T0), s(S_T1), op=Alu.mult)
        v.tensor_tensor(s(S_Y0C), s(S_Y0S), HLb, op=Alu.max)
        v.tensor_tensor(s(S_Y0C), s(S_Y0C), HHb, op=Alu.min)
        v.tensor_tensor(s(S_Y1C), s(S_Y1S), HLb, op=Alu.max)
        v.tensor_tensor(s(S_Y1C), s(S_Y1C), HHb, op=Alu.min)

        v.tensor_tensor(s(S_TXS), dx, KXb, op=Alu.add)
        v.tensor_scalar(s(S_TXS), s(S_TXS), 0.0, None, Alu.max)
        v.tensor_scalar(s(S_X0S), s(S_TXS), 8388607.5, 8388608.0,
                        Alu.add, Alu.subtract)
        v.tensor_tensor(s(S_WX), s(S_TXS), s(S_X0S), op=Alu.subtract)
        v.tensor_scalar(s(S_X1S), s(S_X0S), 1.0, None, Alu.add)
        v.tensor_scalar(s(S_XB), s(S_X0S), 16.0, None, Alu.max)
        v.tensor_scalar(s(S_XB), s(S_XB), 142.0, None, Alu.min)
        v.tensor_scalar(s(S_XB1), s(S_XB), 1.0, None, Alu.add)
        # slot weights: as_m = (1-wx)*[x0==xb+m] + wx*[x1==xb+m]
        v.tensor_scalar(s(S_AWX), s(S_WX), -1.0, 1.0, Alu.mult, Alu.add)
        v.tensor_tensor(s(S_T0), s(S_X0S), s(S_XB), op=Alu.is_equal)
        v.tensor_tensor(s(S_T1), s(S_X1S), s(S_XB), op=Alu.is_equal)
        v.tensor_tensor(s(S_T0), s(S_AWX), s(S_T0), op=Alu.mult)
        v.tensor_tensor(s(S_T1), s(S_WX), s(S_T1), op=Alu.mult)
        v.tensor_tensor(s(S_AS0), s(S_T0), s(S_T1), op=Alu.add)
        v.tensor_tensor(s(S_T0), s(S_X0S), s(S_XB1), op=Alu.is_equal)
        v.tensor_tensor(s(S_T1), s(S_X1S), s(S_XB1), op=Alu.is_equal)
        v.tensor_tensor(s(S_T0), s(S_AWX), s(S_T0), op=Alu.mult)
        v.tensor_tensor(s(S_T1), s(S_WX), s(S_T1), op=Alu.mult)
        v.tensor_tensor(s(S_AS1), s(S_T0), s(S_T1), op=Alu.add)
        # y weights with validity and mask folded in
        v.tensor_scalar(s(S_AWY), s(S_WY), -1.0, 1.0, Alu.mult, Alu.add)
        v.tensor_tensor(s(S_WY0M), s(S_AWY), s(S_V0), op=Alu.mult)
        v.tensor_tensor(s(S_WY0M), s(S_WY0M), s(S_MSK), op=Alu.mult)
        v.tensor_tensor(s(S_WY1M), s(S_WY), s(S_V1), op=Alu.mult)
        v.tensor_tensor(s(S_WY1M), s(S_WY1M), s(S_MSK), op=Alu.mult)
        v.tensor_tensor(W4[:, 0], s(S_WY0M), s(S_AS0), op=Alu.mult)
        v.tensor_tensor(W4[:, 1], s(S_WY0M), s(S_AS1), op=Alu.mult)
        v.tensor_tensor(W4[:, 2], s(S_WY1M), s(S_AS0), op=Alu.mult)
        v.tensor_tensor(W4[:, 3], s(S_WY1M), s(S_AS1), op=Alu.mult)
        # gather unit index = y0c*128 + xb + (128*(h-16-ylo) - 16)
        v.tensor_tensor(s(S_ADR), s(S_XB), HOFb, op=Alu.add)
        v.scalar_tensor_tensor(s(S_I0F), s(S_Y0C), 128.0, s(S_ADR),
                               Alu.mult, Alu.add)
        v.scalar_tensor_tensor(s(S_I1F), s(S_Y1C), 128.0, s(S_ADR),
                               Alu.mult, Alu.add)
        i0v = s(S_I0F).rearrange("p (g r) k -> p g r k", r=GG)
        i1v = s(S_I1F).rearrange("p (g r) k -> p g r k", r=GG)
        v.tensor_copy(IAL[:, :, :, 0, :].rearrange("p g k r -> p g r k"), i0v)
        v.tensor_copy(IAL[:, :, :, 1, :].rearrange("p g k r -> p g r k"), i1v)

        # ---- wrap indices into the 16-partition gather layout + replicate ----
        for sw in range(8):
            src = IAL[16 * sw:16 * (sw + 1)].rearrange("p g k c r -> p (g k c r)")
            nc.sync.dma_start(WR[0:16, :, :, :, :, sw], src)
        nc.sync.dma_start(WR[16:32], WR[0:16])
        nc.sync.dma_start(WR[32:64], WR[0:32])
        nc.sync.dma_start(WR[64:128], WR[0:64])

        # ---- gather + wk scale + MAC + pointwise per group ----
        # the SWDGE descriptor ring holds 128 entries and each gather needs
        # num_idxs/16 + 1, so split each group's 9216 indices into 5 chunks
        for gg in range(NG):
            GT = gpool.tile([128, K2 * 2 * GG, 256], DT.float16, tag="gt",
                            name="gt")
            src = bass.AP(tensor=xs, offset=0, ap=[[128, SLAB_U], [1, 256]])
            idxs = WR[:, gg].rearrange("p k c r s -> p (k c r s)")
            for ck in range(K2):
                sl0, nsl = ck * 64, 64
                nc.gpsimd.dma_gather(GT[:, sl0 // 8:(sl0 + nsl) // 8, :],
                                     src, idxs[:, sl0:sl0 + nsl],
                                     nsl * 16, nidx_regs[nsl], 256,
                                     elem_step=128, queue_num=gidx[0] % 4)
                gidx[0] += 1
            # apply depthwise weight wk[c,k] (2x-rate DVE tensor_tensor)
            for k in range(K2):
                wkv = WKR[:, k, :]
                wkb = bass.AP(tensor=wkv.tensor, offset=wkv.offset,
                              ap=[list(wkv.ap[0]), [0, 2 * GG], [1, 256]])
                gv = GT[:, k * 2 * GG:(k + 1) * 2 * GG, :]
                v.tensor_tensor(gv, gv, wkb, op=Alu.mult)

            TR4 = tr_ps.tile([128, GG, 128], DT.float32, tag="tr", name="tr")
            for rr in range(GG):
                rb = gg * GG + rr

                def gslice(k, c2, s2):
                    return GT[:, (k * 2 + c2) * GG + rr,
                              s2 * 128:(s2 + 1) * 128]

                ACCd = apool.tile([128, 128], DT.float16, tag="accd")
                first = True
                for (k, c2, s2) in dve_terms:
                    g = gslice(k, c2, s2)
                    wsc = W4[:, c2 * 2 + s2, rb, k:k + 1]
                    if first:
                        v.tensor_scalar(ACCd[:], g, wsc, None, Alu.mult)
                        first = False
                    else:
                        v.scalar_tensor_tensor(ACCd[:], g, wsc, ACCd[:],
                                               Alu.mult, Alu.add)
                # transpose the accumulator into [c, w]
                nc.tensor.matmul(TR4[:, rr, :], ACCd[:], IDN[:],
                                 start=True, stop=True)

            RT4 = rpool.tile([128, GG, 128], DT.float16, tag="rt4")
            nc.scalar.activation(RT4[:], TR4[:], AF.Copy)
            OUTS = opool.tile([128, 2, GG, 128], DT.float16, tag="outs")
            PW = pw_ps.tile([128, 2, GG, 128], DT.float32, tag="pw",
                            name="pw")
            rt_flat = RT4[:].rearrange("p g w -> p (g w)")
            for oh in range(2):
                nc.tensor.matmul(PW[:, oh], WPW[:, oh * 128:(oh + 1) * 128],
                                 rt_flat, start=True, stop=True)
                nc.scalar.activation(OUTS[:, oh], PW[:, oh], AF.Identity,
                                     bias=BPW[:, oh:oh + 1])
            r0 = bt * RB + gg * GG
            nc.sync.dma_start(out_v[:, :, r0:r0 + GG, :], OUTS[:])


# ---------------- host side ----------------

def host_prep(inputs):
    x = np.asarray(inputs["x"], np.float32)
    w_off = np.asarray(inputs["w_off"], np.float32)
    b_off = np.asarray(inputs["b_off"], np.float32)
    w_dw = np.asarray(inputs["w_dw"], np.float32)
    b_dw = np.asarray(inputs["b_dw"], np.float32)
    w_pw = np.asarray(inputs["w_pw"], np.float32)
    b_pw = np.asarray(inputs["b_pw"], np.float32)

    wk = w_dw.reshape(C, K2)
    woff_p = np.ascontiguousarray(
        w_off.transpose(1, 2, 3, 0).reshape(C, K2 * 27)).astype(np.float16)
    wpw_p = np.ascontiguousarray(w_pw.T).astype(np.float16)
    idn = np.eye(128, dtype=np.float16)
    bpw_eff = (b_pw + w_pw @ b_dw).astype(np.float32)

    # wk replicated across partitions, per (k, slot, c)
    wkrow = np.tile(wk.T[:, None, :], (1, 2, 1)).reshape(-1)  # [K2*256]
    wkr = np.ascontiguousarray(
        np.broadcast_to(wkrow, (128, K2 * 256))).astype(np.float16)

    ky = (np.arange(K2) // 3 - 1).astype(np.float32)
    kx = (np.arange(K2) % 3 - 1).astype(np.float32)

    # pixel-major fp16 image per batch
    xts = [np.ascontiguousarray(x[b].transpose(1, 2, 0)).astype(np.float16)
           .reshape(H * W, C) for b in range(B)]

    in_maps = []
    for core in range(8):
        b, half = core // 2, core % 2
        r0 = half * ROWS
        ylo = 0 if half == 0 else H - SLAB_ROWS
        xcp = np.zeros([C, 66, 130], np.float32)
        lo, hi = max(r0 - 1, 0), min(r0 + 65, H)
        xcp[:, lo - (r0 - 1): hi - (r0 - 1), 1:129] = x[b][:, lo:hi, :]
        xcp = xcp.astype(np.float16).reshape(C, 66 * 130)

        xsp = np.zeros([SLAB_U + 1, 128], np.float16)
        xsp[0:SLAB_U] = xts[b][ylo * 128:(ylo + SLAB_ROWS) * 128]

        hvec = (r0 + np.arange(ROWS)).astype(np.float32)
        cons = np.zeros([128, CONS_W], np.float32)
        cons[:, 0:9] = ky[None, :] + 16.0
        cons[:, 9:18] = kx[None, :] + 16.0 + np.arange(128, dtype=np.float32)[:, None]
        cons[:, 18:82] = (ylo + 16.0 - hvec)[None, :]
        cons[:, 82:146] = (min(143.0, ylo + SLAB_ROWS + 15.0) - hvec)[None, :]
        cons[:, 146:210] = (128.0 * (hvec - 16.0 - ylo) - 16.0)[None, :]
        cons[:, 210:212] = bpw_eff.reshape(2, 128).T
        cons[:, 212:239] = b_off[None, :]

        in_maps.append({
            "xc": xcp, "xs": xsp, "woff": woff_p, "wpw": wpw_p,
            "idn": idn, "wkr": wkr, "cons": cons,
        })
    return in_maps


def assemble(results):
    out = np.zeros([B, O, H, W], np.float32)
    for core, r in enumerate(results):
        b, half = core // 2, core % 2
        out[b, :, half * ROWS:(half + 1) * ROWS, :] = \
            r["out"].astype(np.float32)
    return out


# ---- single-sync-wait legalization (inlined) ----
_doc = """Legalize BIR for walrus builds that allow only ONE sync wait per
instruction: hoist extra waits onto same-engine NOPs inserted immediately
before the offending instruction."""
import copy

def _make_nop(nc, engine):
    nop = nc.engines[engine].nop(nofuse=True).ins
    # the builder appended it to nc.cur_bb; steal it from wherever it landed
    for f in nc.m.functions:
        for bb in f.blocks:
            il = bb.instructions
            if il and il[-1].name == nop.name:
                il.pop()
                bb.instructions = il
                return nop
    raise RuntimeError("freshly built nop not found")

def legalize_single_wait(nc):
    n_split = 0
    for f in nc.m.functions:
        for bb in f.blocks:
            insts = bb.instructions
            if not any(i.sync_info and len(i.sync_info.on_wait) > 1 for i in insts):
                continue
            out = []
            for inst in insts:
                si = inst.sync_info
                if si and len(si.on_wait) > 1:
                    waits = list(si.on_wait)
                    for w in waits[:-1]:
                        nop = _make_nop(nc, inst.engine)
                        nsi = copy.deepcopy(si)
                        nsi.on_wait = [w]
                        nsi.on_update = []
                        nop.sync_info = nsi
                        out.append(nop)
                    si.on_wait = [waits[-1]]
                    n_split += 1
                out.append(inst)
            bb.instructions = out
    return n_split


_CACHED_NC = None


def kernel(**inputs):
    global _CACHED_NC
    from concourse import bass_utils
    in_maps = host_prep(inputs)
    if _CACHED_NC is None:
        _CACHED_NC = build_nc()
    res = bass_utils.run_bass_kernel_spmd(_CACHED_NC, in_maps,
                                          core_ids=list(range(8)))
    return assemble(res.results)
